# revision 9
# baseline (speedup 1.0000x reference)
"""Trainium2 Bass kernel for a 2-channel diffusion-reaction PDE step.

Computes, for state = [U; V] on a 4096x4096 grid with constant boundary pads:
    dUdt = a*lap(U) + U - U^3 - V - k
    dVdt = b*lap(V) + U - V
with a = sigmoid(a_org)*0.01, etc., dx = 0.1 (so a*inv_dx2 = sigmoid(a_org)).

Strategy (8 cores, 512 rows/core, all on-chip data fp16; rel tolerance is
2e-2, measured pipeline error ~1.3e-3):
  * Channel-interleaved row tiles: each SBUF tile holds 64 U-rows (parts
    0:64) and the same 64 V-rows (parts 64:128), including the 1-row halo.
    One 128x128 weight then computes BOTH channels' vertical taps AND the
    cross terms (-V into dU, +U into dV) in a single matmul per PSUM bank:
      mm1 (W_A): vertical stencil + diagonal (+1/-1 terms folded) + cross
      mmL/mmR (C_I): horizontal taps via +-1 column-shifted moving slices
      mm_t3 (T_I): subtracts t3 = U^3 (fp16 tile) from the U half only
  * Cubic path: sq = x*x, t3 = sq*x on VectorE (fp16 2x mode, some sq ops
    on GpSimd).  V-half rows of t3 hold V^3; T_I's zero columns drop them.
  * PSUM: two 4-bank tiles per row tile; matmuls target 512-wide bank
    slices; evacuation in large-FD chunks (amortizes op overhead):
    ScalarE activation(Identity, bias=kvec) + VectorE tensor_scalar,
    bias = -k on U rows, 0 on V rows; fp16 out.
  * I/O fp16 both ways; output stored tile-major [9,126,4096] so each
    store DMA spreads over all 16 SDMA engines (a [62,...] store only
    uses 8; a DRAM-contiguous store only 2).  Host does the final
    reshuffle + fp32 upcast.
"""

import numpy as np

import concourse.bass as bass
import concourse.mybir as mybir
from concourse import bacc
from concourse.tile import TileContext
from concourse.bass_utils import run_bass_kernel_spmd

NX, NY = 4096, 4096
NCORES = 8
RPC = NX // NCORES       # 512 rows per core
W = NY + 2               # padded width (left/right BC columns)
CT = 512                 # col-tile width (one PSUM bank of fp32)
NCT = NY // CT           # 8 col tiles
# 9 row windows per core; window t outputs rows R0S[t]..R0S[t]+61 and reads
# rows R0S[t]-1..R0S[t]+62 (64 rows/channel).  Last window overlaps so all
# windows are uniform 64-row reads with no zero padding.
R0S = [0, 62, 124, 186, 248, 310, 372, 434, 450]
NRT = len(R0S)

f32 = mybir.dt.float32
f16 = mybir.dt.float16
ALU = mybir.AluOpType
ACTF = mybir.ActivationFunctionType

# weight tile column layout ([128, 384] fp16)
W_A = 0      # cols   0:128  vertical taps + diagonal + cross terms
W_CI = 128   # cols 128:256  horizontal tap coefficient (shifted identity)
W_TI = 256   # cols 256:384  -1 selector for t3 into the U half

_BUILD_CACHE = {}


def _build_nc():
    if "nc" in _BUILD_CACHE:
        return _BUILD_CACHE["nc"]

    nc = bacc.Bacc(None, target_bir_lowering=False)

    uv_in = nc.dram_tensor("uv_in", [NRT, 128, W], f16, kind="ExternalInput")
    wts = nc.dram_tensor("wts", [128, 384], f16, kind="ExternalInput")
    kvec = nc.dram_tensor("kvec", [128, 1], f32, kind="ExternalInput")
    out = nc.dram_tensor("out", [NRT, 126, NY], f16, kind="ExternalOutput")

    with TileContext(nc) as tc:
        with tc.tile_pool(name="wp", bufs=1) as wp, \
             tc.tile_pool(name="inp", bufs=3) as inp, \
             tc.tile_pool(name="sqp", bufs=2) as sqp, \
             tc.tile_pool(name="t3p", bufs=2) as t3p, \
             tc.tile_pool(name="outp", bufs=2) as outp, \
             tc.tile_pool(name="psp", bufs=2, space="PSUM") as psp:

            w_t = wp.tile([128, 384], f16, tag="w")
            nc.sync.dma_start(out=w_t, in_=wts[:, :])
            kv_t = wp.tile([128, 1], f32, tag="kv")
            nc.sync.dma_start(out=kv_t, in_=kvec[:, :])

            for t in range(NRT):
                x = inp.tile([128, W], f16, tag="in")
                nc.sync.dma_start(out=x, in_=uv_in[t, :, :])

                # cubic path: t3 = x^3 (fp16, full tile; V half discarded
                # later by T_I's zero weight columns)
                sq = sqp.tile([128, W], f16, tag="sq")
                nc.vector.tensor_mul(sq, x, x)
                t3 = t3p.tile([128, W], f16, tag="t3")
                nc.vector.tensor_mul(t3, sq, x)

                # two 4-bank psum tiles; matmuls hit 512-wide bank slices
                psA = psp.tile([128, 4 * CT], f32, tag="ps", name=f"psA_{t}")
                psB = psp.tile([128, 4 * CT], f32, tag="ps", name=f"psB_{t}")

                def bank(j):
                    ps = psA if j < 4 else psB
                    c0 = CT * (j % 4)
                    return ps[:, c0:c0 + CT]

                for j in range(NCT):
                    nc.tensor.matmul(bank(j), w_t[:, W_A:W_A + 128],
                                     x[:, CT * j + 1:CT * j + 1 + CT],
                                     start=True, stop=False)
                for j in range(NCT):
                    nc.tensor.matmul(bank(j), w_t[:, W_CI:W_CI + 128],
                                     x[:, CT * j:CT * j + CT],
                                     start=False, stop=False)
                for j in range(NCT):
                    nc.tensor.matmul(bank(j), w_t[:, W_CI:W_CI + 128],
                                     x[:, CT * j + 2:CT * j + 2 + CT],
                                     start=False, stop=False)
                for j in range(NCT):
                    nc.tensor.matmul(bank(j), w_t[:, W_TI:W_TI + 128],
                                     t3[:, CT * j + 1:CT * j + 1 + CT],
                                     start=False, stop=True)

                # evacuate in large-FD chunks on ScalarE; store each half
                # as soon as its evac completes
                o = outp.tile([128, NY], f16, tag="out")
                nc.scalar.activation(o[:, 0:4 * CT], psA, ACTF.Identity,
                                     bias=kv_t[:, 0:1])
                nc.scalar.dma_start(out=out[t, :, 0:4 * CT],
                                    in_=o[0:126, 0:4 * CT])
                nc.scalar.activation(o[:, 4 * CT:8 * CT], psB,
                                     ACTF.Identity, bias=kv_t[:, 0:1])
                nc.scalar.dma_start(out=out[t, :, 4 * CT:8 * CT],
                                    in_=o[0:126, 4 * CT:8 * CT])

    nc.compile()
    _BUILD_CACHE["nc"] = nc
    return nc


def _sigmoid64(x):
    return 1.0 / (1.0 + np.exp(-np.float64(x)))


def _make_weights(c1, c1v):
    wts = np.zeros((128, 384), dtype=np.float32)
    q = np.arange(62)
    du = np.float32(np.float16(-4.0 * np.float64(c1) + 1.0))
    dv = np.float32(np.float16(-4.0 * np.float64(c1v) - 1.0))
    # W_A columns 0:62 -> dU rows; columns 64:126 -> dV rows
    wts[q, W_A + q] = c1              # U up-neighbor
    wts[q + 1, W_A + q] = du          # U center: -4*c1 + 1
    wts[q + 2, W_A + q] = c1          # U down-neighbor
    wts[64 + q + 1, W_A + q] = -1.0   # cross: -V
    wts[64 + q, W_A + 64 + q] = c1v
    wts[64 + q + 1, W_A + 64 + q] = dv
    wts[64 + q + 2, W_A + 64 + q] = c1v
    wts[q + 1, W_A + 64 + q] = 1.0    # cross: +U
    # C_I: horizontal tap (same shifted-diagonal shape for both halves)
    wts[q + 1, W_CI + q] = c1
    wts[64 + q + 1, W_CI + 64 + q] = c1v
    # T_I: subtract t3 from U half only
    wts[q + 1, W_TI + q] = -1.0
    return wts.astype(np.float16)


def _make_in_maps(state, bc, a_org, b_org, k_org):
    c1 = np.float32(_sigmoid64(a_org))       # a * inv_dx2 == sigmoid(a_org)
    c1v = np.float32(_sigmoid64(b_org))
    k = np.float32(_sigmoid64(k_org) * 0.01)

    wts = _make_weights(c1, c1v)
    kvec = np.zeros((128, 1), dtype=np.float32)
    kvec[0:62] = -k

    st = np.asarray(state)[0]                # [2, NX, NY] fp32
    bc = np.asarray(bc, dtype=np.float32)

    # full padded fp16 grids for both channels: [2, NX+2, NY+2]
    pad = np.empty((2, NX + 2, NY + 2), dtype=np.float16)
    pad[:, 1:-1, 1:-1] = st.astype(np.float16)
    for ch in range(2):
        pad[ch, :, 0] = np.float16(bc[0, ch, 0])     # left
        pad[ch, :, -1] = np.float16(bc[0, ch, 1])    # right
        pad[ch, 0, :] = np.float16(bc[0, ch, 2])     # top
        pad[ch, -1, :] = np.float16(bc[0, ch, 3])    # bottom

    in_maps = []
    for c in range(NCORES):
        g0 = RPC * c
        uv = np.empty((NRT, 128, W), dtype=np.float16)
        for t, r0 in enumerate(R0S):
            uv[t, 0:64] = pad[0, g0 + r0:g0 + r0 + 64]
            uv[t, 64:128] = pad[1, g0 + r0:g0 + r0 + 64]
        in_maps.append({"uv_in": uv, "wts": wts, "kvec": kvec})
    return in_maps


def _run(in_maps, trace=False, **kwargs):
    nc = _build_nc()
    return run_bass_kernel_spmd(nc, in_maps, list(range(NCORES)),
                                trace=trace, **kwargs)


def kernel(state, bc, a_org, b_org, k_org):
    in_maps = _make_in_maps(state, bc, a_org, b_org, k_org)
    res = _run(in_maps).results
    full = np.empty((1, 2, NX, NY), dtype=np.float32)
    for c in range(NCORES):
        o = res[c]["out"].astype(np.float32)     # [NRT, 126, NY]
        g0 = RPC * c
        for t, r0 in enumerate(R0S):
            q0 = 0 if t < NRT - 1 else 46
            nr = 62 if t < NRT - 1 else 16
            full[0, 0, g0 + r0 + q0:g0 + r0 + q0 + nr] = o[t, q0:q0 + nr]
            full[0, 1, g0 + r0 + q0:g0 + r0 + q0 + nr] = o[t, 64 + q0:64 + q0 + nr]
    return full


# revision 11
# speedup vs baseline: 1.1807x; 1.1807x over previous
"""Trainium2 Bass kernel for a 2-channel diffusion-reaction PDE step.

Computes, for state = [U; V] on a 4096x4096 grid with constant boundary pads:
    dUdt = a*lap(U) + U - U^3 - V - k
    dVdt = b*lap(V) + U - V
with a = sigmoid(a_org)*0.01, etc., dx = 0.1 (so a*inv_dx2 = sigmoid(a_org)).

Strategy (8 cores, 512 rows/core, all on-chip data fp16; rel tolerance is
2e-2, measured pipeline error ~1.3e-3):
  * Channel-interleaved row tiles: each SBUF tile holds 64 U-rows (parts
    0:64) and the same 64 V-rows (parts 64:128), including the 1-row halo.
    One 128x128 weight then computes BOTH channels' vertical taps AND the
    cross terms (-V into dU, +U into dV) in a single matmul per PSUM bank:
      mm1 (W_A): vertical stencil + diagonal (+1/-1 terms folded) + cross
      mmL/mmR (C_I): horizontal taps via +-1 column-shifted moving slices
      mm_t3 (T_I): subtracts t3 = U^3 (fp16 tile) from the U half only
  * Cubic path: sq = x*x, t3 = sq*x on VectorE (fp16 2x mode, some sq ops
    on GpSimd).  V-half rows of t3 hold V^3; T_I's zero columns drop them.
  * PSUM: two 4-bank tiles per row tile; matmuls target 512-wide bank
    slices; evacuation in large-FD chunks (amortizes op overhead):
    ScalarE activation(Identity, bias=kvec) + VectorE tensor_scalar,
    bias = -k on U rows, 0 on V rows; fp16 out.
  * I/O fp16 both ways; output stored tile-major [9,126,4096] so each
    store DMA spreads over all 16 SDMA engines (a [62,...] store only
    uses 8; a DRAM-contiguous store only 2).  Host does the final
    reshuffle + fp32 upcast.
"""

import numpy as np

import concourse.bass as bass
import concourse.mybir as mybir
from concourse import bacc
from concourse.tile import TileContext
from concourse.bass_utils import run_bass_kernel_spmd

NX, NY = 4096, 4096
NCORES = 8
RPC = NX // NCORES       # 512 rows per core
W = NY + 2               # padded width (left/right BC columns)
CT = 512                 # col-tile width (one PSUM bank of fp32)
NCT = NY // CT           # 8 col tiles
# 9 row windows per core; window t outputs rows R0S[t]..R0S[t]+61 and reads
# rows R0S[t]-1..R0S[t]+62 (64 rows/channel).  Last window overlaps so all
# windows are uniform 64-row reads with no zero padding.
R0S = [0, 62, 124, 186, 248, 310, 372, 434, 450]
NRT = len(R0S)

f32 = mybir.dt.float32
f16 = mybir.dt.float16
ALU = mybir.AluOpType
ACTF = mybir.ActivationFunctionType

# weight tile column layout ([128, 384] fp16)
W_A = 0      # cols   0:128  vertical taps + diagonal + cross terms
W_CI = 128   # cols 128:256  horizontal tap coefficient (shifted identity)
W_TI = 256   # cols 256:384  -1 selector for t3 into the U half

_BUILD_CACHE = {}


def _build_nc():
    if "nc" in _BUILD_CACHE:
        return _BUILD_CACHE["nc"]

    nc = bacc.Bacc(None, target_bir_lowering=False)

    uv_in = nc.dram_tensor("uv_in", [NRT, 128, W], f16, kind="ExternalInput")
    wts = nc.dram_tensor("wts", [128, 384], f16, kind="ExternalInput")
    kvec = nc.dram_tensor("kvec", [128, 1], f32, kind="ExternalInput")
    out = nc.dram_tensor("out", [NRT, 126, NY], f16, kind="ExternalOutput")

    with TileContext(nc) as tc:
        with tc.tile_pool(name="wp", bufs=1) as wp, \
             tc.tile_pool(name="inp", bufs=3) as inp, \
             tc.tile_pool(name="sqp", bufs=2) as sqp, \
             tc.tile_pool(name="t3p", bufs=2) as t3p, \
             tc.tile_pool(name="outp", bufs=2) as outp, \
             tc.tile_pool(name="psp", bufs=2, space="PSUM") as psp:

            w_t = wp.tile([128, 384], f16, tag="w")
            nc.sync.dma_start(out=w_t, in_=wts[:, :])
            kv_t = wp.tile([128, 1], f32, tag="kv")
            nc.sync.dma_start(out=kv_t, in_=kvec[:, :])

            for t in range(NRT):
                x = inp.tile([128, W], f16, tag="in")
                nc.sync.dma_start(out=x, in_=uv_in[t, :, :])

                # cubic path: t3 = x^3 (fp16, full tile; V half discarded
                # later by T_I's zero weight columns)
                sq = sqp.tile([128, W], f16, tag="sq")
                nc.vector.tensor_mul(sq, x, x)
                t3 = t3p.tile([128, W], f16, tag="t3")
                nc.vector.tensor_mul(t3, sq, x)

                # two 4-bank psum tiles; matmuls hit 512-wide bank slices
                psA = psp.tile([128, 4 * CT], f32, tag="ps", name=f"psA_{t}")
                psB = psp.tile([128, 4 * CT], f32, tag="ps", name=f"psB_{t}")

                def bank(j):
                    ps = psA if j < 4 else psB
                    c0 = CT * (j % 4)
                    return ps[:, c0:c0 + CT]

                for j in range(NCT):
                    nc.tensor.matmul(bank(j), w_t[:, W_A:W_A + 128],
                                     x[:, CT * j + 1:CT * j + 1 + CT],
                                     start=True, stop=False)
                for j in range(NCT):
                    nc.tensor.matmul(bank(j), w_t[:, W_CI:W_CI + 128],
                                     x[:, CT * j:CT * j + CT],
                                     start=False, stop=False)
                for j in range(NCT):
                    nc.tensor.matmul(bank(j), w_t[:, W_CI:W_CI + 128],
                                     x[:, CT * j + 2:CT * j + 2 + CT],
                                     start=False, stop=False)
                for j in range(NCT):
                    nc.tensor.matmul(bank(j), w_t[:, W_TI:W_TI + 128],
                                     t3[:, CT * j + 1:CT * j + 1 + CT],
                                     start=False, stop=True)

                # evacuate in large-FD chunks on ScalarE; store each half
                # as soon as its evac completes
                o = outp.tile([128, NY], f16, tag="out")
                nc.scalar.activation(o[:, 0:4 * CT], psA, ACTF.Identity,
                                     bias=kv_t[:, 0:1])
                nc.scalar.dma_start(out=out[t, :, 0:4 * CT],
                                    in_=o[0:126, 0:4 * CT])
                nc.scalar.activation(o[:, 4 * CT:8 * CT], psB,
                                     ACTF.Identity, bias=kv_t[:, 0:1])
                nc.scalar.dma_start(out=out[t, :, 4 * CT:8 * CT],
                                    in_=o[0:126, 4 * CT:8 * CT])

    nc.compile()
    _BUILD_CACHE["nc"] = nc
    return nc


def _sigmoid64(x):
    return 1.0 / (1.0 + np.exp(-np.float64(x)))


def _make_weights(c1, c1v):
    wts = np.zeros((128, 384), dtype=np.float32)
    q = np.arange(62)
    du = np.float32(np.float16(-4.0 * np.float64(c1) + 1.0))
    dv = np.float32(np.float16(-4.0 * np.float64(c1v) - 1.0))
    # W_A columns 0:62 -> dU rows; columns 64:126 -> dV rows
    wts[q, W_A + q] = c1              # U up-neighbor
    wts[q + 1, W_A + q] = du          # U center: -4*c1 + 1
    wts[q + 2, W_A + q] = c1          # U down-neighbor
    wts[64 + q + 1, W_A + q] = -1.0   # cross: -V
    wts[64 + q, W_A + 64 + q] = c1v
    wts[64 + q + 1, W_A + 64 + q] = dv
    wts[64 + q + 2, W_A + 64 + q] = c1v
    wts[q + 1, W_A + 64 + q] = 1.0    # cross: +U
    # C_I: horizontal tap (same shifted-diagonal shape for both halves)
    wts[q + 1, W_CI + q] = c1
    wts[64 + q + 1, W_CI + 64 + q] = c1v
    # T_I: subtract t3 from U half only
    wts[q + 1, W_TI + q] = -1.0
    return wts.astype(np.float16)


def _make_in_maps(state, bc, a_org, b_org, k_org):
    c1 = np.float32(_sigmoid64(a_org))       # a * inv_dx2 == sigmoid(a_org)
    c1v = np.float32(_sigmoid64(b_org))
    k = np.float32(_sigmoid64(k_org) * 0.01)

    wts = _make_weights(c1, c1v)
    kvec = np.zeros((128, 1), dtype=np.float32)
    kvec[0:62] = -k

    st = np.asarray(state)[0]                # [2, NX, NY] fp32
    bc = np.asarray(bc, dtype=np.float32)

    # full padded fp16 grids for both channels: [2, NX+2, NY+2]
    pad = np.empty((2, NX + 2, NY + 2), dtype=np.float16)
    pad[:, 1:-1, 1:-1] = st.astype(np.float16)
    for ch in range(2):
        pad[ch, :, 0] = np.float16(bc[0, ch, 0])     # left
        pad[ch, :, -1] = np.float16(bc[0, ch, 1])    # right
        pad[ch, 0, :] = np.float16(bc[0, ch, 2])     # top
        pad[ch, -1, :] = np.float16(bc[0, ch, 3])    # bottom

    in_maps = []
    for c in range(NCORES):
        g0 = RPC * c
        uv = np.empty((NRT, 128, W), dtype=np.float16)
        for t, r0 in enumerate(R0S):
            uv[t, 0:64] = pad[0, g0 + r0:g0 + r0 + 64]
            uv[t, 64:128] = pad[1, g0 + r0:g0 + r0 + 64]
        in_maps.append({"uv_in": uv, "wts": wts, "kvec": kvec})
    return in_maps


def _run(in_maps, trace=False, **kwargs):
    nc = _build_nc()
    return run_bass_kernel_spmd(nc, in_maps, list(range(NCORES)),
                                trace=trace, **kwargs)


def kernel(state, bc, a_org, b_org, k_org):
    in_maps = _make_in_maps(state, bc, a_org, b_org, k_org)
    res = _run(in_maps).results
    full = np.empty((1, 2, NX, NY), dtype=np.float32)
    for c in range(NCORES):
        o = res[c]["out"].astype(np.float32)     # [NRT, 126, NY]
        g0 = RPC * c
        for t, r0 in enumerate(R0S):
            q0 = 0 if t < NRT - 1 else 46
            nr = 62 if t < NRT - 1 else 16
            full[0, 0, g0 + r0 + q0:g0 + r0 + q0 + nr] = o[t, q0:q0 + nr]
            full[0, 1, g0 + r0 + q0:g0 + r0 + q0 + nr] = o[t, 64 + q0:64 + q0 + nr]
    return full
